# revision 6
# baseline (speedup 1.0000x reference)
"""Tensor-parallel MultiHeadAttention (QKV + RoPE + GQA causal SDPA + dense)
for 8 Trainium2 NeuronCores.

Sharding (TP as in TPMultiHeadAttention): core d owns query heads {2d, 2d+1}
and the single kv head d//2 (kv heads replicated across core pairs), plus the
matching 256 columns of the dense projection. Each core produces a full-shape
partial output; the all-reduce is a host-side sum over the 8 partials.

Per-core device pipeline (all matmul feeds in fp16 -> full PE rate AND
fast weight load, which f32/f32r stationary operands disable):
  0. ~14 dummy warmup matmuls on a memset scratch tile flip the PE's HAM
     clock gate to full speed (2.4 GHz) while the first DMAs stream in;
     without them the first ~28us of QKV run at the cold 1.2 GHz clock.
  1. qkv^T = W_shard @ x^T            -> [f=512, s=2048] (f on partitions)
  2. RoPE on q,k via a permutation matmul (rotate_half) + DVE combine;
     softmax scale folded into the q-side weights
  3. S^T[sk, sq] = k'^T.T @ q'  per 128-row sk tile, per 512-col sq chunk;
     diagonal sk tiles are computed only over their causally visible query
     sub-range (N in {512, 384, 256, 128}).  P^T = exp(S^T - ln 1024) on
     ScalarE (the bias keeps fp16 exp sums < 300 and cancels in the softmax
     normalization); each diagonal tile's partial 128-wide triangle is
     masked multiplicatively after exp.
  4. ctx^T[d, sq] += v_nat[sk,d].T @ P^T  (v transposed once via PE).
     Softmax denominators: P^T tiles are summed into two parallel fp16
     accumulator chains on the otherwise-idle GpSimd engine, column-summed
     with a ones-vector matmul, reciprocal'd (fast custom-DVE op),
     gpsimd-broadcast over partitions, and multiplied into ctx^T.
  5. out[s, e] += ctx^T.T @ wd^T  (accumulate over the 2 local heads);
     the 16 output tiles of chunk c are emitted interleaved into chunk
     c+1's attention tile stream so the PE always has independent work
     while ScalarE (exp, the Phase-B pacing stage) catches up.  Partial
     outputs are stored fp16 (halves the writeback); host sums in f32.
"""

import numpy as np

B, S, E = 1, 2048, 2048
H, KVH, D = 16, 4, 128
NCORES = 8
P = 128
FD = 512            # matmul moving free dim == one fp32 PSUM bank
NE = E // P         # 16 contraction tiles over the embedding dim
NSC = S // FD       # 4 sequence chunks
NST = S // P        # 16 sequence tiles
FLOC = 4 * P        # local fused qkv rows per core (2 q heads + k + v)
ROPE_BASE = 10000.0
# causally visible query sub-range start for diagonal sk tile o
DIAG_START = (0, 128, 256, 384)
LOG_BIAS = -float(np.log(1024.0))   # exp bias; cancels in normalization
NWARM = 14

LAST_RESULT = None
_BASS_CACHE = None


def _rope_tables():
    inv = 1.0 / (ROPE_BASE ** (np.arange(0, D, 2, dtype=np.float64) / D))
    t = np.arange(S, dtype=np.float64)
    freqs = np.outer(t, inv)
    emb = np.concatenate([freqs, freqs], axis=-1)  # [S, D]
    return np.cos(emb), np.sin(emb)


def _host_constants():
    cos, sin = _rope_tables()
    consts = {}
    consts["cosr"] = np.ascontiguousarray(cos.T.astype(np.float16))
    consts["sinr"] = np.ascontiguousarray(sin.T.astype(np.float16))
    # [128, 128] lower-triangle(r <= c); multiplies every diagonal tile's
    # leading 128 columns after exp
    r_idx = np.arange(P)[:, None]
    c_idx = np.arange(P)[None, :]
    consts["maskm"] = np.ascontiguousarray((r_idx <= c_idx).astype(np.float16))
    # rotate_half as a matmul: rot = M @ q (in [d, s] layout); pass M.T as lhsT
    M = np.zeros((P, P), np.float16)
    half = D // 2
    M[np.arange(half), np.arange(half) + half] = -1.0
    M[np.arange(half) + half, np.arange(half)] = 1.0
    consts["protT"] = np.ascontiguousarray(M.T)
    consts["ebias"] = np.full((P, 1), LOG_BIAS, np.float32)
    consts["ident"] = np.eye(P, dtype=np.float32)
    consts["ones"] = np.ones((P, 1), np.float16)
    return consts


def _build_bass():
    import concourse.mybir as mybir
    import concourse.tile as tile
    from concourse import bacc

    f32 = mybir.dt.float32
    f16 = mybir.dt.float16
    Exp = mybir.ActivationFunctionType.Exp

    nc = bacc.Bacc(None, target_bir_lowering=False, name="mha_tp8")
    # x pre-tiled on host to [eo, sc, p, f] so every tile DMA reads a fully
    # contiguous 128KB block (strided reads cap DMA engines at ~11GB/s)
    xTt = nc.dram_tensor("xTt", [NE, NSC, P, FD], f16, kind="ExternalInput")
    wqkvT = nc.dram_tensor("wqkvT", [E, FLOC], f16, kind="ExternalInput")
    wdT = nc.dram_tensor("wdT", [2 * P, S], f16, kind="ExternalInput")
    cosr = nc.dram_tensor("cosr", [P, S], f16, kind="ExternalInput")
    sinr = nc.dram_tensor("sinr", [P, S], f16, kind="ExternalInput")
    maskm = nc.dram_tensor("maskm", [P, P], f16, kind="ExternalInput")
    protT = nc.dram_tensor("protT", [P, P], f16, kind="ExternalInput")
    ident = nc.dram_tensor("ident", [P, P], f32, kind="ExternalInput")
    ones = nc.dram_tensor("ones", [P, 1], f16, kind="ExternalInput")
    ebias = nc.dram_tensor("ebias", [P, 1], f32, kind="ExternalInput")
    # output tiled [c, st, eo, p, f]; host reassembles to [s, e]
    out = nc.dram_tensor("out", [NSC, 4, 4, P, FD], f16, kind="ExternalOutput")

    with tile.TileContext(nc) as tc:
        with tc.tile_pool(name="const", bufs=1) as const:
            # Weight slices go out on the gpsimd ring, the x tiles on the
            # sync ring (two descriptor queues feeding in parallel);
            # tables/mask (scalar ring) and dense weights stay off both.
            w_sb = const.tile([P, NE, FLOC], f16, name="w_sb")
            pr = const.tile([P, P], f16, name="pr")
            idn = const.tile([P, P], f32, name="idn")
            on = const.tile([P, 1], f16, name="on")
            eb = const.tile([P, 1], f32, name="eb")
            warm = const.tile([P, FD], f16, name="warm")

            cq = const.tile([P, S], f16, name="cq")
            sq_t = const.tile([P, S], f16, name="sq_t")
            mk = const.tile([P, P], f16, name="mk")
            wd_sb = const.tile([P, 2, S], f16, name="wd_sb")

            qr = const.tile([P, 2, S], f16, name="qr")
            kr = const.tile([P, S], f16, name="kr")
            vT = const.tile([P, S], f32, name="vT")
            vn = const.tile([P, NST, P], f16, name="vn")

            # ---- Phase A: fused QKV projection + RoPE + v transpose ----
            with tc.tile_pool(name="xs_p", bufs=16) as xpool, \
                 tc.tile_pool(name="ps_qkv", bufs=1, space="PSUM") as pqkv, \
                 tc.tile_pool(name="ps_rot", bufs=2, space="PSUM") as prot_p, \
                 tc.tile_pool(name="ps_vt", bufs=2, space="PSUM") as pvt, \
                 tc.tile_pool(name="rtmp", bufs=3) as rtmp:
                # HAM warmup: PE busy-work with no DMA dependency
                nc.vector.memset(warm, 0.0)
                for i in range(NWARM):
                    wp = prot_p.tile([P, FD], f32, tag="rot", name=f"warm_{i}")
                    nc.tensor.matmul(wp, lhsT=warm[:, :P], rhs=warm,
                                     start=True, stop=True)
                nc.scalar.dma_start(cq, cosr[:, :])
                nc.scalar.dma_start(sq_t, sinr[:, :])
                nc.scalar.dma_start(mk, maskm[:, :])
                nc.scalar.dma_start(wd_sb, wdT.rearrange("(h p) e -> p h e", p=P))
                for sc in range(NSC):
                    ssl = slice(sc * FD, (sc + 1) * FD)
                    psums = [
                        pqkv.tile([P, FD], f32, tag=f"qkv{f}", name=f"ps_qkv{f}_{sc}")
                        for f in range(4)
                    ]
                    for eo in range(NE):
                        if sc == 0:
                            nc.gpsimd.dma_start(
                                w_sb[:, eo, :], wqkvT[eo * P:(eo + 1) * P, :]
                            )
                        xs = xpool.tile([P, FD], f16, tag="xs", name=f"xs_{sc}_{eo}")
                        nc.sync.dma_start(xs, xTt[eo, sc])
                        if sc == 0 and eo == 0:
                            nc.sync.dma_start(pr, protT[:, :])
                            nc.sync.dma_start(idn, ident[:, :])
                            nc.sync.dma_start(on, ones[:, :])
                            nc.sync.dma_start(eb, ebias[:, :])
                        for f in range(4):
                            nc.tensor.matmul(
                                psums[f],
                                lhsT=w_sb[:, eo, f * P:(f + 1) * P],
                                rhs=xs,
                                start=(eo == 0),
                                stop=(eo == NE - 1),
                            )
                    # psum-freeing copies on ScalarE (idle in this phase) so
                    # the next chunk's matmuls get their banks back quickly
                    for f in range(4):
                        pt = psums[f]
                        if f == 3:
                            nc.scalar.copy(vT[:, ssl], pt)
                            continue
                        cos_t, sin_t = cq, sq_t
                        dst = qr[:, f, ssl] if f < 2 else kr[:, ssl]
                        qt = rtmp.tile([P, FD], f16, tag="qt", name=f"qt_{sc}_{f}")
                        nc.scalar.copy(qt, pt)
                        rp = prot_p.tile([P, FD], f32, tag="rot", name=f"rot_{sc}_{f}")
                        nc.tensor.matmul(rp, lhsT=pr, rhs=qt, start=True, stop=True)
                        tt = rtmp.tile([P, FD], f16, tag="tt", name=f"tt_{sc}_{f}")
                        nc.vector.tensor_mul(tt, rp, sin_t[:, ssl])
                        nc.vector.tensor_mul(dst, qt, cos_t[:, ssl])
                        nc.vector.tensor_add(dst, dst, tt)
                    for jj in range(4):
                        j = 4 * sc + jj
                        vp = pvt.tile([P, P], f32, tag="vt", name=f"vt_{j}")
                        nc.tensor.transpose(vp, vT[:, j * P:(j + 1) * P], idn)
                        nc.scalar.copy(vn[:, j, :], vp)

            # ---- Phase B: attention + dense, per 512-query chunk ----
            # dense(c) output tiles are interleaved one-per-attention-tile
            # into attn(c+1)'s stream: independent PE work next to every
            # exp-paced stall.
            with tc.tile_pool(name="ps_s", bufs=3, space="PSUM") as ps_s, \
                 tc.tile_pool(name="ps_ctx", bufs=2, space="PSUM") as ps_ctx, \
                 tc.tile_pool(name="ps_r", bufs=1, space="PSUM") as ps_r, \
                 tc.tile_pool(name="ps_o", bufs=2, space="PSUM") as ps_o, \
                 tc.tile_pool(name="pt_p", bufs=4) as ptp, \
                 tc.tile_pool(name="acc_p", bufs=3) as accp, \
                 tc.tile_pool(name="rb_p", bufs=3) as rbp, \
                 tc.tile_pool(name="ctx_p", bufs=3) as ctxp, \
                 tc.tile_pool(name="out_p", bufs=4) as outp:
                all_csb = {}

                def dense_tile(c, st, eo, k):
                    op = ps_o.tile([P, FD], f32, tag="o", name=f"o_{c}_{st}_{eo}")
                    for h in range(2):
                        nc.tensor.matmul(
                            op,
                            lhsT=all_csb[(c, h)][:, st * P:(st + 1) * P],
                            rhs=wd_sb[:, h, eo * FD:(eo + 1) * FD],
                            start=(h == 0), stop=(h == 1),
                        )
                    ot = outp.tile([P, FD], f16, tag="ot", name=f"ot_{c}_{st}_{eo}")
                    # ScalarE is exp-bound in this phase: 1 in 4 release
                    # copies goes there, the rest to DVE
                    if k % 4 == 0:
                        nc.scalar.copy(ot, op)
                    else:
                        nc.vector.tensor_copy(ot, op)
                    nc.sync.dma_start(out[c, st, eo], ot)

                def dense_quanta(c):
                    if c < 0:
                        return iter(())
                    return iter([
                        (c, st, eo, 4 * st + eo) for st in range(4) for eo in range(4)
                    ])

                def emit_attn(c, dq):
                    qbase = c * FD
                    nj = 4 * c + 4
                    two_chain = c >= 1
                    ctxps, accs = {}, {}
                    for h in range(2):
                        ctxps[h] = ps_ctx.tile([P, FD], f32, tag="ctx", name=f"ctx_{c}_{h}")
                        acc_a = accp.tile([P, FD], f16, tag=f"acca{h}", name=f"acca_{c}_{h}")
                        acc_b = (
                            accp.tile([P, FD], f16, tag=f"accb{h}", name=f"accb_{c}_{h}")
                            if two_chain else None
                        )
                        accs[h] = (acc_a, acc_b)
                        for j in range(nj):
                            o = j - 4 * c
                            so = DIAG_START[o] if o >= 0 else 0
                            n = FD - so
                            sp = ps_s.tile([P, FD], f32, tag="s", name=f"s_{c}_{h}_{j}")
                            nc.tensor.matmul(
                                sp[:, :n],
                                lhsT=kr[:, j * P:(j + 1) * P],
                                rhs=qr[:, h, qbase + so: qbase + FD],
                                start=True, stop=True,
                            )
                            pt = ptp.tile([P, FD], f16, tag="pt", name=f"pt_{c}_{h}_{j}")
                            nc.scalar.activation(pt[:, :n], sp[:, :n], Exp, bias=eb[:, :])
                            if o >= 0:
                                # mask the leading 128-wide partial triangle
                                nc.vector.tensor_mul(pt[:, :P], pt[:, :P], mk)
                            acc = acc_b if (two_chain and j % 2) else acc_a
                            if j < (2 if two_chain else 1):
                                nc.gpsimd.tensor_copy(acc, pt)
                            else:
                                nc.gpsimd.tensor_add(
                                    acc[:, so:], acc[:, so:], pt[:, :n],
                                )
                            nc.tensor.matmul(
                                ctxps[h][:, so:],
                                lhsT=vn[:, j, :],
                                rhs=pt[:, :n],
                                start=(j == 0), stop=(j == nj - 1),
                            )
                            for q in (next(dq, None),):
                                if q is not None:
                                    dense_tile(*q)
                    # softmax tails after both heads' tile loops, so the PE
                    # stream never head-of-line blocks on an acc chain
                    for h in range(2):
                        acc_a, acc_b = accs[h]
                        rp_ = ps_r.tile([1, FD], f32, tag="r", name=f"r_{c}_{h}")
                        if two_chain:
                            nc.tensor.matmul(rp_, lhsT=on, rhs=acc_a, start=True, stop=False)
                            nc.tensor.matmul(rp_, lhsT=on, rhs=acc_b, start=False, stop=True)
                        else:
                            nc.tensor.matmul(rp_, lhsT=on, rhs=acc_a, start=True, stop=True)
                        rec = rbp.tile([1, FD], f32, tag="rec", name=f"rec_{c}_{h}")
                        nc.vector.reciprocal_approx_fast(rec, rp_)
                        rb = rbp.tile([P, FD], f32, tag="rb", name=f"rb_{c}_{h}")
                        nc.gpsimd.partition_broadcast(rb, rec)
                        ct = ctxp.tile([P, FD], f16, tag=f"ctx{h}", name=f"csb_{c}_{h}")
                        nc.vector.tensor_mul(ct, ctxps[h], rb)
                        all_csb[(c, h)] = ct
                    # any dense quanta not yet emitted (c=0's stream)
                    for q in dq:
                        dense_tile(*q)

                emit_attn(0, dense_quanta(-1))
                emit_attn(1, dense_quanta(0))
                emit_attn(2, dense_quanta(1))
                emit_attn(3, dense_quanta(2))
                for q in dense_quanta(3):
                    dense_tile(*q)
    nc.compile()
    return nc


def make_in_maps(x, w_qkv, w_dense):
    x = np.asarray(x, np.float32).reshape(S, E)
    w_qkv = np.asarray(w_qkv, np.float32)
    w_dense = np.asarray(w_dense, np.float32)
    # x^T tiled to [eo, sc, p, f] so device DMAs are contiguous
    xTt = np.ascontiguousarray(
        x.T.reshape(NE, P, NSC, FD).transpose(0, 2, 1, 3)
    ).astype(np.float16)
    consts = _host_constants()
    in_maps = []
    scale = np.float32(1.0 / np.sqrt(D))
    for d in range(NCORES):
        g = d // 2
        wq = w_qkv[2 * d * P:(2 * d + 2) * P] * scale
        wk = w_qkv[H * D + g * P: H * D + (g + 1) * P]
        wv = w_qkv[H * D + KVH * D + g * P: H * D + KVH * D + (g + 1) * P]
        wqkvT_d = np.ascontiguousarray(
            np.concatenate([wq, wk, wv], 0).T
        ).astype(np.float16)
        wdT_d = np.ascontiguousarray(
            w_dense[:, 2 * d * P:(2 * d + 2) * P].T
        ).astype(np.float16)
        m = {"xTt": xTt, "wqkvT": wqkvT_d, "wdT": wdT_d}
        m.update(consts)
        in_maps.append(m)
    return in_maps


def kernel(x, w_qkv, w_dense):
    global LAST_RESULT, _BASS_CACHE
    from concourse.bass_utils import run_bass_kernel_spmd

    in_maps = make_in_maps(x, w_qkv, w_dense)
    if _BASS_CACHE is None:
        _BASS_CACHE = _build_bass()
    res = run_bass_kernel_spmd(_BASS_CACHE, in_maps, core_ids=list(range(NCORES)))
    LAST_RESULT = res
    # sum partials over cores, then untile [c, st, eo, p, f] -> [s, e]
    acc = np.zeros((NSC, 4, 4, P, FD), np.float32)
    for r in res.results:
        acc += r["out"]
    full = acc.transpose(0, 1, 3, 2, 4).reshape(S, E)
    return np.ascontiguousarray(full).reshape(B, S, E)


# revision 7
# speedup vs baseline: 1.4248x; 1.4248x over previous
"""Tensor-parallel MultiHeadAttention (QKV + RoPE + GQA causal SDPA + dense)
for 8 Trainium2 NeuronCores.

Sharding (TP as in TPMultiHeadAttention): core d owns query heads {2d, 2d+1}
and the single kv head d//2 (kv heads replicated across core pairs), plus the
matching 256 columns of the dense projection. Each core produces a full-shape
partial output; the all-reduce is a host-side sum over the 8 partials.

Per-core device pipeline (all matmul feeds in fp16 -> full PE rate AND
fast weight load, which f32/f32r stationary operands disable):
  0. ~14 dummy warmup matmuls on a memset scratch tile flip the PE's HAM
     clock gate to full speed (2.4 GHz) while the first DMAs stream in;
     without them the first ~28us of QKV run at the cold 1.2 GHz clock.
  1. qkv^T = W_shard @ x^T            -> [f=512, s=2048] (f on partitions)
  2. RoPE on q,k via a permutation matmul (rotate_half) + DVE combine;
     softmax scale folded into the q-side weights
  3. S^T[sk, sq] = k'^T.T @ q'  per 128-row sk tile, per 512-col sq chunk;
     diagonal sk tiles are computed only over their causally visible query
     sub-range (N in {512, 384, 256, 128}).  P^T = exp(S^T - ln 1024) on
     ScalarE (the bias keeps fp16 exp sums < 300 and cancels in the softmax
     normalization); each diagonal tile's partial 128-wide triangle is
     masked multiplicatively after exp.
  4. ctx^T[d, sq] += v_nat[sk,d].T @ P^T  (v transposed once via PE).
     Softmax denominators: P^T tiles are summed into two parallel fp16
     accumulator chains on DVE (2x packed mode), column-summed
     with a ones-vector matmul, reciprocal'd (fast custom-DVE op),
     gpsimd-broadcast over partitions, and multiplied into ctx^T.
  5. out[s, e] += ctx^T.T @ wd^T  (accumulate over the 2 local heads);
     the 16 output tiles of chunk c are emitted interleaved into chunk
     c+1's attention tile stream so the PE always has independent work
     while ScalarE (exp, the Phase-B pacing stage) catches up.  Partial
     outputs are stored fp16 (halves the writeback); host sums in f32.
"""

import numpy as np

B, S, E = 1, 2048, 2048
H, KVH, D = 16, 4, 128
NCORES = 8
P = 128
FD = 512            # matmul moving free dim == one fp32 PSUM bank
NE = E // P         # 16 contraction tiles over the embedding dim
NSC = S // FD       # 4 sequence chunks
NST = S // P        # 16 sequence tiles
FLOC = 4 * P        # local fused qkv rows per core (2 q heads + k + v)
ROPE_BASE = 10000.0
# causally visible query sub-range start for diagonal sk tile o
DIAG_START = (0, 128, 256, 384)
LOG_BIAS = -float(np.log(1024.0))   # exp bias; cancels in normalization
NWARM = 14

LAST_RESULT = None
_BASS_CACHE = None


def _rope_tables():
    inv = 1.0 / (ROPE_BASE ** (np.arange(0, D, 2, dtype=np.float64) / D))
    t = np.arange(S, dtype=np.float64)
    freqs = np.outer(t, inv)
    emb = np.concatenate([freqs, freqs], axis=-1)  # [S, D]
    return np.cos(emb), np.sin(emb)


def _host_constants():
    cos, sin = _rope_tables()
    consts = {}
    consts["cosr"] = np.ascontiguousarray(cos.T.astype(np.float16))
    consts["sinr"] = np.ascontiguousarray(sin.T.astype(np.float16))
    # [128, 128] lower-triangle(r <= c); multiplies every diagonal tile's
    # leading 128 columns after exp
    r_idx = np.arange(P)[:, None]
    c_idx = np.arange(P)[None, :]
    consts["maskm"] = np.ascontiguousarray((r_idx <= c_idx).astype(np.float16))
    # rotate_half as a matmul: rot = M @ q (in [d, s] layout); pass M.T as lhsT
    M = np.zeros((P, P), np.float16)
    half = D // 2
    M[np.arange(half), np.arange(half) + half] = -1.0
    M[np.arange(half) + half, np.arange(half)] = 1.0
    consts["protT"] = np.ascontiguousarray(M.T)
    consts["ebias"] = np.full((P, 1), LOG_BIAS, np.float32)
    consts["ident"] = np.eye(P, dtype=np.float32)
    consts["ones"] = np.ones((P, 1), np.float16)
    return consts


def _build_bass():
    import concourse.mybir as mybir
    import concourse.tile as tile
    from concourse import bacc

    f32 = mybir.dt.float32
    f16 = mybir.dt.float16
    Exp = mybir.ActivationFunctionType.Exp

    nc = bacc.Bacc(None, target_bir_lowering=False, name="mha_tp8")
    # x pre-tiled on host to [eo, sc, p, f] so every tile DMA reads a fully
    # contiguous 128KB block (strided reads cap DMA engines at ~11GB/s)
    xTt = nc.dram_tensor("xTt", [NE, NSC, P, FD], f16, kind="ExternalInput")
    wqkvT = nc.dram_tensor("wqkvT", [E, FLOC], f16, kind="ExternalInput")
    wdT = nc.dram_tensor("wdT", [2 * P, S], f16, kind="ExternalInput")
    cosr = nc.dram_tensor("cosr", [P, S], f16, kind="ExternalInput")
    sinr = nc.dram_tensor("sinr", [P, S], f16, kind="ExternalInput")
    maskm = nc.dram_tensor("maskm", [P, P], f16, kind="ExternalInput")
    protT = nc.dram_tensor("protT", [P, P], f16, kind="ExternalInput")
    ident = nc.dram_tensor("ident", [P, P], f32, kind="ExternalInput")
    ones = nc.dram_tensor("ones", [P, 1], f16, kind="ExternalInput")
    ebias = nc.dram_tensor("ebias", [P, 1], f32, kind="ExternalInput")
    # output tiled [c, st, eo, p, f]; host reassembles to [s, e]
    out = nc.dram_tensor("out", [NSC, 4, 4, P, FD], f16, kind="ExternalOutput")

    with tile.TileContext(nc) as tc:
        with tc.tile_pool(name="const", bufs=1) as const:
            # Weight slices go out on the gpsimd ring, the x tiles on the
            # sync ring (two descriptor queues feeding in parallel);
            # tables/mask (scalar ring) and dense weights stay off both.
            w_sb = const.tile([P, NE, FLOC], f16, name="w_sb")
            pr = const.tile([P, P], f16, name="pr")
            idn = const.tile([P, P], f32, name="idn")
            on = const.tile([P, 1], f16, name="on")
            eb = const.tile([P, 1], f32, name="eb")
            warm = const.tile([P, FD], f16, name="warm")

            cq = const.tile([P, S], f16, name="cq")
            sq_t = const.tile([P, S], f16, name="sq_t")
            mk = const.tile([P, P], f16, name="mk")
            wd_sb = const.tile([P, 2, S], f16, name="wd_sb")

            qr = const.tile([P, 2, S], f16, name="qr")
            kr = const.tile([P, S], f16, name="kr")
            vT = const.tile([P, S], f32, name="vT")
            vn = const.tile([P, NST, P], f16, name="vn")

            # ---- Phase A: fused QKV projection + RoPE + v transpose ----
            with tc.tile_pool(name="xs_p", bufs=16) as xpool, \
                 tc.tile_pool(name="ps_qkv", bufs=1, space="PSUM") as pqkv, \
                 tc.tile_pool(name="ps_rot", bufs=2, space="PSUM") as prot_p, \
                 tc.tile_pool(name="ps_vt", bufs=2, space="PSUM") as pvt, \
                 tc.tile_pool(name="rtmp", bufs=3) as rtmp:
                # HAM warmup: PE busy-work with no DMA dependency
                nc.vector.memset(warm, 0.0)
                for i in range(NWARM):
                    wp = prot_p.tile([P, FD], f32, tag="rot", name=f"warm_{i}")
                    nc.tensor.matmul(wp, lhsT=warm[:, :P], rhs=warm,
                                     start=True, stop=True)
                nc.scalar.dma_start(cq, cosr[:, :])
                nc.scalar.dma_start(sq_t, sinr[:, :])
                nc.scalar.dma_start(mk, maskm[:, :])
                nc.scalar.dma_start(wd_sb, wdT.rearrange("(h p) e -> p h e", p=P))
                for sc in range(NSC):
                    ssl = slice(sc * FD, (sc + 1) * FD)
                    psums = [
                        pqkv.tile([P, FD], f32, tag=f"qkv{f}", name=f"ps_qkv{f}_{sc}")
                        for f in range(4)
                    ]
                    for eo in range(NE):
                        if sc == 0:
                            nc.gpsimd.dma_start(
                                w_sb[:, eo, :], wqkvT[eo * P:(eo + 1) * P, :]
                            )
                        xs = xpool.tile([P, FD], f16, tag="xs", name=f"xs_{sc}_{eo}")
                        nc.sync.dma_start(xs, xTt[eo, sc])
                        if sc == 0 and eo == 0:
                            nc.sync.dma_start(pr, protT[:, :])
                            nc.sync.dma_start(idn, ident[:, :])
                            nc.sync.dma_start(on, ones[:, :])
                            nc.sync.dma_start(eb, ebias[:, :])
                        for f in range(4):
                            nc.tensor.matmul(
                                psums[f],
                                lhsT=w_sb[:, eo, f * P:(f + 1) * P],
                                rhs=xs,
                                start=(eo == 0),
                                stop=(eo == NE - 1),
                            )
                    # psum-freeing copies on ScalarE (idle in this phase) so
                    # the next chunk's matmuls get their banks back quickly
                    for f in range(4):
                        pt = psums[f]
                        if f == 3:
                            nc.scalar.copy(vT[:, ssl], pt)
                            continue
                        cos_t, sin_t = cq, sq_t
                        dst = qr[:, f, ssl] if f < 2 else kr[:, ssl]
                        qt = rtmp.tile([P, FD], f16, tag="qt", name=f"qt_{sc}_{f}")
                        nc.scalar.copy(qt, pt)
                        rp = prot_p.tile([P, FD], f32, tag="rot", name=f"rot_{sc}_{f}")
                        nc.tensor.matmul(rp, lhsT=pr, rhs=qt, start=True, stop=True)
                        tt = rtmp.tile([P, FD], f16, tag="tt", name=f"tt_{sc}_{f}")
                        nc.vector.tensor_mul(tt, rp, sin_t[:, ssl])
                        nc.vector.tensor_mul(dst, qt, cos_t[:, ssl])
                        nc.vector.tensor_add(dst, dst, tt)
                    for jj in range(4):
                        j = 4 * sc + jj
                        vp = pvt.tile([P, P], f32, tag="vt", name=f"vt_{j}")
                        nc.tensor.transpose(vp, vT[:, j * P:(j + 1) * P], idn)
                        nc.scalar.copy(vn[:, j, :], vp)

            # ---- Phase B: attention + dense, per 512-query chunk ----
            # dense(c) output tiles are interleaved one-per-attention-tile
            # into attn(c+1)'s stream: independent PE work next to every
            # exp-paced stall.
            with tc.tile_pool(name="ps_s", bufs=3, space="PSUM") as ps_s, \
                 tc.tile_pool(name="ps_ctx", bufs=2, space="PSUM") as ps_ctx, \
                 tc.tile_pool(name="ps_r", bufs=1, space="PSUM") as ps_r, \
                 tc.tile_pool(name="ps_o", bufs=2, space="PSUM") as ps_o, \
                 tc.tile_pool(name="pt_p", bufs=4) as ptp, \
                 tc.tile_pool(name="acc_p", bufs=3) as accp, \
                 tc.tile_pool(name="rb_p", bufs=3) as rbp, \
                 tc.tile_pool(name="ctx_p", bufs=3) as ctxp, \
                 tc.tile_pool(name="out_p", bufs=4) as outp:
                all_csb = {}

                def dense_tile(c, st, eo, k):
                    op = ps_o.tile([P, FD], f32, tag="o", name=f"o_{c}_{st}_{eo}")
                    for h in range(2):
                        nc.tensor.matmul(
                            op,
                            lhsT=all_csb[(c, h)][:, st * P:(st + 1) * P],
                            rhs=wd_sb[:, h, eo * FD:(eo + 1) * FD],
                            start=(h == 0), stop=(h == 1),
                        )
                    ot = outp.tile([P, FD], f16, tag="ot", name=f"ot_{c}_{st}_{eo}")
                    # ScalarE is exp-bound in this phase: 1 in 4 release
                    # copies goes there, the rest to DVE
                    if k % 4 == 0:
                        nc.scalar.copy(ot, op)
                    else:
                        nc.vector.tensor_copy(ot, op)
                    nc.sync.dma_start(out[c, st, eo], ot)

                def dense_quanta(c):
                    if c < 0:
                        return iter(())
                    return iter([
                        (c, st, eo, 4 * st + eo) for st in range(4) for eo in range(4)
                    ])

                def emit_attn(c, dq):
                    qbase = c * FD
                    nj = 4 * c + 4
                    two_chain = c >= 1
                    ctxps, accs = {}, {}
                    for h in range(2):
                        ctxps[h] = ps_ctx.tile([P, FD], f32, tag="ctx", name=f"ctx_{c}_{h}")
                        acc_a = accp.tile([P, FD], f16, tag=f"acca{h}", name=f"acca_{c}_{h}")
                        acc_b = (
                            accp.tile([P, FD], f16, tag=f"accb{h}", name=f"accb_{c}_{h}")
                            if two_chain else None
                        )
                        accs[h] = (acc_a, acc_b)
                        for j in range(nj):
                            o = j - 4 * c
                            so = DIAG_START[o] if o >= 0 else 0
                            n = FD - so
                            sp = ps_s.tile([P, FD], f32, tag="s", name=f"s_{c}_{h}_{j}")
                            nc.tensor.matmul(
                                sp[:, :n],
                                lhsT=kr[:, j * P:(j + 1) * P],
                                rhs=qr[:, h, qbase + so: qbase + FD],
                                start=True, stop=True,
                            )
                            pt = ptp.tile([P, FD], f16, tag="pt", name=f"pt_{c}_{h}_{j}")
                            nc.scalar.activation(pt[:, :n], sp[:, :n], Exp, bias=eb[:, :])
                            if o >= 0:
                                # mask the leading 128-wide partial triangle
                                nc.vector.tensor_mul(pt[:, :P], pt[:, :P], mk)
                            acc = acc_b if (two_chain and j % 2) else acc_a
                            if j < (2 if two_chain else 1):
                                nc.vector.tensor_copy(acc, pt)
                            else:
                                nc.vector.tensor_add(
                                    acc[:, so:], acc[:, so:], pt[:, :n],
                                )
                            nc.tensor.matmul(
                                ctxps[h][:, so:],
                                lhsT=vn[:, j, :],
                                rhs=pt[:, :n],
                                start=(j == 0), stop=(j == nj - 1),
                            )
                            for q in (next(dq, None),):
                                if q is not None:
                                    dense_tile(*q)
                    # softmax tails after both heads' tile loops, so the PE
                    # stream never head-of-line blocks on an acc chain
                    for h in range(2):
                        acc_a, acc_b = accs[h]
                        rp_ = ps_r.tile([1, FD], f32, tag="r", name=f"r_{c}_{h}")
                        if two_chain:
                            nc.tensor.matmul(rp_, lhsT=on, rhs=acc_a, start=True, stop=False)
                            nc.tensor.matmul(rp_, lhsT=on, rhs=acc_b, start=False, stop=True)
                        else:
                            nc.tensor.matmul(rp_, lhsT=on, rhs=acc_a, start=True, stop=True)
                        rec = rbp.tile([1, FD], f32, tag="rec", name=f"rec_{c}_{h}")
                        nc.vector.reciprocal_approx_fast(rec, rp_)
                        rb = rbp.tile([P, FD], f32, tag="rb", name=f"rb_{c}_{h}")
                        nc.gpsimd.partition_broadcast(rb, rec)
                        ct = ctxp.tile([P, FD], f16, tag=f"ctx{h}", name=f"csb_{c}_{h}")
                        nc.vector.tensor_mul(ct, ctxps[h], rb)
                        all_csb[(c, h)] = ct
                    # any dense quanta not yet emitted (c=0's stream)
                    for q in dq:
                        dense_tile(*q)

                emit_attn(0, dense_quanta(-1))
                emit_attn(1, dense_quanta(0))
                emit_attn(2, dense_quanta(1))
                emit_attn(3, dense_quanta(2))
                for q in dense_quanta(3):
                    dense_tile(*q)
    nc.compile()
    return nc


def make_in_maps(x, w_qkv, w_dense):
    x = np.asarray(x, np.float32).reshape(S, E)
    w_qkv = np.asarray(w_qkv, np.float32)
    w_dense = np.asarray(w_dense, np.float32)
    # x^T tiled to [eo, sc, p, f] so device DMAs are contiguous
    xTt = np.ascontiguousarray(
        x.T.reshape(NE, P, NSC, FD).transpose(0, 2, 1, 3)
    ).astype(np.float16)
    consts = _host_constants()
    in_maps = []
    scale = np.float32(1.0 / np.sqrt(D))
    for d in range(NCORES):
        g = d // 2
        wq = w_qkv[2 * d * P:(2 * d + 2) * P] * scale
        wk = w_qkv[H * D + g * P: H * D + (g + 1) * P]
        wv = w_qkv[H * D + KVH * D + g * P: H * D + KVH * D + (g + 1) * P]
        wqkvT_d = np.ascontiguousarray(
            np.concatenate([wq, wk, wv], 0).T
        ).astype(np.float16)
        wdT_d = np.ascontiguousarray(
            w_dense[:, 2 * d * P:(2 * d + 2) * P].T
        ).astype(np.float16)
        m = {"xTt": xTt, "wqkvT": wqkvT_d, "wdT": wdT_d}
        m.update(consts)
        in_maps.append(m)
    return in_maps


def kernel(x, w_qkv, w_dense):
    global LAST_RESULT, _BASS_CACHE
    from concourse.bass_utils import run_bass_kernel_spmd

    in_maps = make_in_maps(x, w_qkv, w_dense)
    if _BASS_CACHE is None:
        _BASS_CACHE = _build_bass()
    res = run_bass_kernel_spmd(_BASS_CACHE, in_maps, core_ids=list(range(NCORES)))
    LAST_RESULT = res
    # sum partials over cores, then untile [c, st, eo, p, f] -> [s, e]
    acc = np.zeros((NSC, 4, 4, P, FD), np.float32)
    for r in res.results:
        acc += r["out"]
    full = acc.transpose(0, 1, 3, 2, 4).reshape(S, E)
    return np.ascontiguousarray(full).reshape(B, S, E)
